# revision 39
# baseline (speedup 1.0000x reference)
"""Trainium2 Bass kernel for single-headed attention.

Problem: nn_Attention_17471926960981
  q,k,v: [4, 2048, 1024] f32; Wq/Wk/Wv: [1024,1024]; bq/bk/bv: [1024] (zeros)
  out = softmax((q@Wq)(k@Wk)^T / sqrt(1024)) @ (v@Wv)   per batch item

Sharding: 8 cores = (batch b in 0..3, seq-half h in 0..1). Each core gets
1024 rows of q for its batch item plus the full k/v of that item and
computes its 1024 output rows independently.

Algebraic restructure (associativity; host does the cheap 1024^3 prep):
  scores = (q Wq)(k Wk)^T = q A k^T          with A = Wq Wk^T (host sgemm)
  out    = P (v Wv)       = (P v) Wv
so the device never projects k or v.

v2 layout (all matmul operands bf16, ~0.5% rel err vs the 2e-2 gate;
PSUM accumulation f32; measured 173.5us vs the 225.6us fp32r baseline):
  1. Q'^T [d, q] = A^T q^T, resident in SBUF (no DRAM spill), computed in
     column sub-chunks sized to the DMA arrival schedule.
  2. S^T computed directly in [k-part, q-free] orientation per 512-wide q
     chunk: stationary = kT d-block, moving = Q'^T. exp() on ACT writes
     P^T straight into the layout the U matmul needs -- no PE transposes,
     no transpose copies. Softmax is shift-invariant and scaled scores are
     O(1): no row-max pass.
  3. Row-sums of P via ap=1 accumulation chains: stationary = P^T block,
     moving = a ones column (memset, no DMA). Lands rs as [q-part, 1] with
     near-zero PE engine time. The four chains share one PSUM bank, and
     matmul start_tensor_calc zeroes the WHOLE bank on real hardware, so
     the bank is zeroed once by a DVE memset and every chain element runs
     start=False (pure accumulate).
  4. U^T [d, q] = v^T P^T; O = U^T.T Wv, normalized by 1/rowsum on the
     way out (ACT/DVE alternating), stored bf16 per 512-half as soon as
     each half's PSUM chain completes (host upcasts to f32).
  5. A 2-matmul PE warm-up on a memset tile pins the tensor-engine
     p-state ramp (an idle PE engine resets it, doubling the cost of the
     next ~3us of dispatches), all input DMA goes on the SP queue in
     exact first-consumption order, and phases are ordered (Q' c0,c1 |
     S0 | U0+rs0 | S1 | O0 | U1+rs1 | O1) so every consumer's inputs are
     produced a full phase ahead -- the PE engine runs gap-free at full
     rate mid-program (busy ~164.3us of a 163.8us matmul floor).

Biases are structurally zero in this problem; kernel() falls back to an
exact numpy path in the (never exercised) case they are nonzero.
"""

import os
import sys

import numpy as np

try:
    import concourse.bass as bass  # noqa: F401
except ImportError:  # pragma: no cover
    sys.path.insert(0, "/opt/trn_rl_repo")

from contextlib import ExitStack

import concourse.bass as bass  # noqa: F401
import concourse.bass_utils as bass_utils
import concourse.mybir as mybir
import concourse.tile as tile
from concourse import bacc

B, S, D = 4, 2048, 1024
P = 128
SQ = S // 2          # q rows per core
SK = S               # kv rows per core
DT = D // P          # 8 d-tiles
KT = SK // P         # 16 k-tiles
NC_ = 8              # cores

F32 = mybir.dt.float32
BF16 = mybir.dt.bfloat16
EXP = mybir.ActivationFunctionType.Exp
INV_SQRT_D = 1.0 / float(np.sqrt(D))

N_WARM = int(os.environ.get("K_WARM", "2"))
MM_BUFS = int(os.environ.get("K_MM_BUFS", "2"))
O_BUFS = int(os.environ.get("K_O_BUFS", "2"))


def _build_program():
    nc = bacc.Bacc(
        "TRN2",
        target_bir_lowering=False,
        debug=False,
        enable_asserts=False,
        num_devices=NC_,
    )
    qst = nc.dram_tensor("qst", (D, SQ), BF16, kind="ExternalInput").ap()
    kst = nc.dram_tensor("kst", (D, SK), BF16, kind="ExternalInput").ap()
    vsn = nc.dram_tensor("vsn", (SK, D), BF16, kind="ExternalInput").ap()
    wa = nc.dram_tensor("wa", (D, D), BF16, kind="ExternalInput").ap()
    wv = nc.dram_tensor("wv", (D, D), BF16, kind="ExternalInput").ap()
    out = nc.dram_tensor("out", (SQ, D), BF16, kind="ExternalOutput").ap()

    q_r = qst.rearrange("(it p) s -> p it s", p=P)
    k_r = kst.rearrange("(it p) s -> p it s", p=P)
    v_r = vsn.rearrange("(st p) d -> p st d", p=P)
    wa_r = wa.rearrange("(t p) n -> p t n", p=P)
    wv_r = wv.rearrange("(t p) n -> p t n", p=P)

    with tile.TileContext(nc) as tc, ExitStack() as ctx:
        sb = ctx.enter_context(tc.tile_pool(name="sb", bufs=1))
        ut_pool = ctx.enter_context(tc.tile_pool(name="ut", bufs=2))
        osb_pool = ctx.enter_context(tc.tile_pool(name="osb", bufs=4))
        mm_ps = ctx.enter_context(tc.tile_pool(name="mm", bufs=MM_BUFS, space="PSUM"))
        o_ps = ctx.enter_context(tc.tile_pool(name="o", bufs=2 * O_BUFS, space="PSUM"))
        rs_ps_pool = ctx.enter_context(tc.tile_pool(name="rs", bufs=1, space="PSUM"))
        warm_ps_pool = ctx.enter_context(tc.tile_pool(name="wm", bufs=1, space="PSUM"))

        qt_in = sb.tile([P, DT, SQ], BF16, tag="qt_in")
        wa_sb = sb.tile([P, DT, D], BF16, tag="wa")
        kt_sb = sb.tile([P, DT, SK], BF16, tag="kt")
        v_sb = sb.tile([P, KT, D], BF16, tag="v")
        wv_sb = sb.tile([P, DT, D], BF16, tag="wv")
        qpt_sb = sb.tile([P, DT, SQ], BF16, tag="qpt")
        pt_sb = sb.tile([P, KT, 512], BF16, tag="pt")
        ones_sb = sb.tile([P, 1], BF16, tag="ones")
        warm_sb = sb.tile([P, 512], BF16, tag="warm")
        rec_sb = sb.tile([P, 8], F32, tag="rec")

        rs_ps = rs_ps_pool.tile([P, 8], F32, tag="rs")
        warm_ps = warm_ps_pool.tile([P, 512], F32, tag="wm")

        # -- warm-up: pins the PE p-state ramp from ~1.2us (gpsimd memset
        # latency; the DVE queue's init drain is longer).
        nc.gpsimd.memset(warm_sb[:], 0.0)
        nc.gpsimd.memset(ones_sb[:], 1.0)
        nc.vector.memset(rs_ps[:], 0.0)
        for i in range(N_WARM):
            col = (i % 2) * 256
            nc.tensor.matmul(
                warm_ps[:, col : col + 256], warm_sb[:, 0:P],
                warm_sb[:, col : col + 256], start=True, stop=True,
            )

        # -- input DMA --
        # Transfers serialize on the DMA engines (~360 GB/s) and the SP
        # (HWDGE) queue is served predictably, so every input goes on SP in
        # exact first-consumption order: wa / qT interleaved to feed the Q'
        # sub-chunks as they start, then kT in S order, then v, then Wv.
        nc.sync.dma_start(wa_sb[:, 0:2, 0:256], wa_r[:, 0:2, 0:256])
        nc.sync.dma_start(qt_in[:, 0:2, 0:256], q_r[:, 0:2, 0:256])
        nc.sync.dma_start(wa_sb[:, 2:4, 0:256], wa_r[:, 2:4, 0:256])
        nc.sync.dma_start(qt_in[:, 2:4, 0:256], q_r[:, 2:4, 0:256])
        nc.sync.dma_start(wa_sb[:, 4:DT, 0:256], wa_r[:, 4:DT, 0:256])
        nc.sync.dma_start(qt_in[:, 4:DT, 0:256], q_r[:, 4:DT, 0:256])
        nc.sync.dma_start(wa_sb[:, :, 256:512], wa_r[:, :, 256:512])
        nc.sync.dma_start(wa_sb[:, :, 512:768], wa_r[:, :, 512:768])
        nc.sync.dma_start(wa_sb[:, :, 768:1024], wa_r[:, :, 768:1024])
        nc.sync.dma_start(qt_in[:, :, 256:512], q_r[:, :, 256:512])
        nc.sync.dma_start(qt_in[:, :, 512:1024], q_r[:, :, 512:1024])
        for g in range(SK // 512):
            nc.sync.dma_start(kt_sb[:, :, g * 512 : (g + 1) * 512],
                              k_r[:, :, g * 512 : (g + 1) * 512])
        nc.sync.dma_start(v_sb[:, 0:8, :], v_r[:, 0:8, :])
        nc.sync.dma_start(v_sb[:, 8:16, :], v_r[:, 8:16, :])
        nc.sync.dma_start(wv_sb[:], wv_r[:])

        # -- Q' phase: Q'^T[jt, cols] = sum_it A[it, jt-block]^T qT[it, cols]
        # in sub-chunks sized to the DMA arrival schedule --
        ncopy = 0
        for sub, (lo, hi) in enumerate(((0, 256), (256, 512), (512, 1024))):
            for jt in range(DT):
                qp = mm_ps.tile([P, 512], F32, tag="mm")
                for it in range(DT):
                    nc.tensor.matmul(
                        qp[:, 0 : hi - lo],
                        wa_sb[:, it, jt * P : (jt + 1) * P],
                        qt_in[:, it, lo:hi],
                        start=(it == 0),
                        stop=(it == DT - 1),
                    )
                dst = qpt_sb[:, jt, lo:hi]
                if ncopy % 2 == 0:
                    nc.scalar.copy(dst, qp[:, 0 : hi - lo])
                else:
                    nc.vector.tensor_copy(dst, qp[:, 0 : hi - lo])
                ncopy += 1

        def s_group(c, st):
            sps = mm_ps.tile([P, 512], F32, tag="mm")
            for it in range(DT):
                nc.tensor.matmul(
                    sps[:],
                    kt_sb[:, it, st * P : (st + 1) * P],
                    qpt_sb[:, it, c * 512 : (c + 1) * 512],
                    start=(it == 0),
                    stop=(it == DT - 1),
                )
            nc.scalar.activation(pt_sb[:, st, :], sps[:], EXP, scale=INV_SQRT_D)

        def rs_elems(c, st):
            # ap=1 accumulation chains: rs[:, c*4+qt] += PT[st][:, qtile]^T @ 1.
            # start=False always: matmul start_tensor_calc zeroes the whole
            # PSUM bank, which would erase the sibling chains sharing this
            # bank -- the bank is zeroed once by a memset instead.
            for qt_i in range(4):
                nc.tensor.matmul(
                    rs_ps[:, c * 4 + qt_i : c * 4 + qt_i + 1],
                    pt_sb[:, st, qt_i * P : (qt_i + 1) * P],
                    ones_sb[:],
                    start=False,
                    stop=(st == KT - 1),
                    skip_group_check=True,
                )

        def u_group(c, dt_i, ut):
            ups = mm_ps.tile([P, 512], F32, tag="mm")
            for st in range(KT):
                nc.tensor.matmul(
                    ups[:],
                    v_sb[:, st, dt_i * P : (dt_i + 1) * P],
                    pt_sb[:, st, :],
                    start=(st == 0),
                    stop=(st == KT - 1),
                )
            dst = ut[:, dt_i, :]
            if dt_i % 2 == 0:
                nc.scalar.copy(dst, ups[:])
            else:
                nc.vector.tensor_copy(dst, ups[:])

        def o_phase(c, ut):
            nc.vector.reciprocal(rec_sb[:, c * 4 : (c + 1) * 4],
                                 rs_ps[:, c * 4 : (c + 1) * 4])
            for qt_i in range(4):
                t = c * 4 + qt_i
                osb = osb_pool.tile([P, D], BF16, tag="osb")
                rec = rec_sb[:, t : t + 1]
                # each 512-half gets its own PSUM tile (finer release) and
                # its normalize+store fires as soon as its chain completes,
                # overlapping the next half's matmuls. Stores alternate
                # queues so the final ones don't sit behind one queue's
                # per-DMA latency backlog.
                for h in range(2):
                    ops = o_ps.tile([P, 512], F32, tag="o")
                    for i in range(DT):
                        nc.tensor.matmul(
                            ops[:],
                            ut[:, i, qt_i * P : (qt_i + 1) * P],
                            wv_sb[:, i, h * 512 : (h + 1) * 512],
                            start=(i == 0),
                            stop=(i == DT - 1),
                        )
                    dst = osb[:, h * 512 : (h + 1) * 512]
                    if (qt_i + h) % 2 == 0:
                        nc.scalar.mul(dst, ops[:], rec)
                        dma_eng = nc.sync
                    else:
                        nc.vector.tensor_scalar_mul(dst, ops[:], rec)
                        dma_eng = nc.scalar if c == 1 else nc.gpsimd
                    dma_eng.dma_start(
                        out[t * P : (t + 1) * P, h * 512 : (h + 1) * 512], dst
                    )

        # -- attention: S0 | U0+rs0 | S1 | O0 | U1+rs1 | O1 --
        ut0 = ut_pool.tile([P, DT, 512], BF16, tag="ut")
        for st in range(KT):
            s_group(0, st)
            if st > 0:
                rs_elems(0, st - 1)
        for dt_i in range(DT):
            u_group(0, dt_i, ut0)
            if dt_i == 0:
                rs_elems(0, KT - 1)

        ut1 = ut_pool.tile([P, DT, 512], BF16, tag="ut")
        for st in range(KT):
            s_group(1, st)
            if st > 0:
                rs_elems(1, st - 1)
        o_phase(0, ut0)
        for dt_i in range(DT):
            u_group(1, dt_i, ut1)
            if dt_i == 0:
                rs_elems(1, KT - 1)
        o_phase(1, ut1)

    nc.compile()
    return nc


_NC_CACHE = {}


def _get_nc():
    if "nc" not in _NC_CACHE:
        _NC_CACHE["nc"] = _build_program()
    return _NC_CACHE["nc"]


def _numpy_fallback(q, k, v, Wq, bq, Wk, bk, Wv, bv):
    out = np.empty((B, S, D), np.float32)
    for b in range(B):
        qp = q[b] @ Wq + bq
        kp = k[b] @ Wk + bk
        vpv = v[b] @ Wv + bv
        s = (qp @ kp.T) * INV_SQRT_D
        s -= s.max(axis=-1, keepdims=True)
        p = np.exp(s)
        p /= p.sum(axis=-1, keepdims=True)
        out[b] = p @ vpv
    return out


def kernel(q, k, v, Wq, bq, Wk, bk, Wv, bv):
    from ml_dtypes import bfloat16

    q = np.asarray(q, np.float32)
    k = np.asarray(k, np.float32)
    v = np.asarray(v, np.float32)
    Wq = np.ascontiguousarray(np.asarray(Wq, np.float32))
    Wk = np.ascontiguousarray(np.asarray(Wk, np.float32))
    Wv = np.ascontiguousarray(np.asarray(Wv, np.float32))
    bq = np.asarray(bq, np.float32)
    bk = np.asarray(bk, np.float32)
    bv = np.asarray(bv, np.float32)

    if np.any(bq) or np.any(bk) or np.any(bv):
        # Never hit for this problem (biases are structurally zero), kept for
        # exactness of the kernel contract.
        return _numpy_fallback(q, k, v, Wq, bq, Wk, bk, Wv, bv)

    nc = _get_nc()
    A = (Wq @ Wk.T).astype(bfloat16)         # scores = q A k^T
    wv_b = Wv.astype(bfloat16)
    kt_full = [np.ascontiguousarray(k[b].T.astype(bfloat16)) for b in range(B)]
    v_full = [np.ascontiguousarray(v[b].astype(bfloat16)) for b in range(B)]
    in_maps = []
    for b in range(B):
        for h in range(2):
            in_maps.append(
                {
                    "qst": np.ascontiguousarray(
                        q[b, h * SQ : (h + 1) * SQ, :].T.astype(bfloat16)
                    ),
                    "kst": kt_full[b],
                    "vsn": v_full[b],
                    "wa": A,
                    "wv": wv_b,
                }
            )

    res = bass_utils.run_bass_kernel_spmd(nc, in_maps, core_ids=list(range(NC_)))

    out = np.empty((B, S, D), np.float32)
    for c, r in enumerate(res.results):
        b, h = divmod(c, 2)
        out[b, h * SQ : (h + 1) * SQ, :] = np.asarray(r["out"], np.float32)
    return out


if __name__ == "__main__":
    rng = np.random.default_rng(0)
    scale = 1.0 / np.sqrt(D)
    inputs = {
        "q": rng.standard_normal((B, S, D)).astype(np.float32),
        "k": rng.standard_normal((B, S, D)).astype(np.float32),
        "v": rng.standard_normal((B, S, D)).astype(np.float32),
        "Wq": (rng.standard_normal((D, D)) * scale).astype(np.float32),
        "bq": np.zeros(D, np.float32),
        "Wk": (rng.standard_normal((D, D)) * scale).astype(np.float32),
        "bk": np.zeros(D, np.float32),
        "Wv": (rng.standard_normal((D, D)) * scale).astype(np.float32),
        "bv": np.zeros(D, np.float32),
    }
    actual = kernel(**inputs)
    expected = _numpy_fallback(**inputs)
    err = np.linalg.norm(actual - expected) / np.linalg.norm(expected)
    print("rel err:", err)


# revision 41
# speedup vs baseline: 1.0050x; 1.0050x over previous
"""Trainium2 Bass kernel for single-headed attention.

Problem: nn_Attention_17471926960981
  q,k,v: [4, 2048, 1024] f32; Wq/Wk/Wv: [1024,1024]; bq/bk/bv: [1024] (zeros)
  out = softmax((q@Wq)(k@Wk)^T / sqrt(1024)) @ (v@Wv)   per batch item

Sharding: 8 cores = (batch b in 0..3, seq-half h in 0..1). Each core gets
1024 rows of q for its batch item plus the full k/v of that item and
computes its 1024 output rows independently.

Algebraic restructure (associativity; host does the cheap 1024^3 prep):
  scores = (q Wq)(k Wk)^T = q A k^T          with A = Wq Wk^T (host sgemm)
  out    = P (v Wv)       = (P v) Wv
so the device never projects k or v.

v2 layout (all matmul operands bf16, ~0.5% rel err vs the 2e-2 gate;
PSUM accumulation f32; measured 173.5us vs the 225.6us fp32r baseline):
  1. Q'^T [d, q] = A^T q^T, resident in SBUF (no DRAM spill), computed in
     column sub-chunks sized to the DMA arrival schedule.
  2. S^T computed directly in [k-part, q-free] orientation per 512-wide q
     chunk: stationary = kT d-block, moving = Q'^T. exp() on ACT writes
     P^T straight into the layout the U matmul needs -- no PE transposes,
     no transpose copies. Softmax is shift-invariant and scaled scores are
     O(1): no row-max pass.
  3. Row-sums of P via ap=1 accumulation chains: stationary = P^T block,
     moving = a ones column (memset, no DMA). Lands rs as [q-part, 1] with
     near-zero PE engine time. The four chains share one PSUM bank, and
     matmul start_tensor_calc zeroes the WHOLE bank on real hardware, so
     the bank is zeroed once by a DVE memset and every chain element runs
     start=False (pure accumulate).
  4. U^T [d, q] = v^T P^T; O = U^T.T Wv, normalized by 1/rowsum on the
     way out (ACT/DVE alternating), stored bf16 per 512-half as soon as
     each half's PSUM chain completes (host upcasts to f32).
  5. A 2-matmul PE warm-up on a memset tile pins the tensor-engine
     p-state ramp (an idle PE engine resets it, doubling the cost of the
     next ~3us of dispatches), all input DMA goes on the SP queue in
     exact first-consumption order, and phases are ordered (Q' c0,c1 |
     S0 | U0+rs0 | S1 | O0 | U1+rs1 | O1) so every consumer's inputs are
     produced a full phase ahead -- the PE engine runs gap-free at full
     rate mid-program (busy ~164.3us of a 163.8us matmul floor).

Biases are structurally zero in this problem; kernel() falls back to an
exact numpy path in the (never exercised) case they are nonzero.
"""

import os
import sys

import numpy as np

try:
    import concourse.bass as bass  # noqa: F401
except ImportError:  # pragma: no cover
    sys.path.insert(0, "/opt/trn_rl_repo")

from contextlib import ExitStack

import concourse.bass as bass  # noqa: F401
import concourse.bass_utils as bass_utils
import concourse.mybir as mybir
import concourse.tile as tile
from concourse import bacc

B, S, D = 4, 2048, 1024
P = 128
SQ = S // 2          # q rows per core
SK = S               # kv rows per core
DT = D // P          # 8 d-tiles
KT = SK // P         # 16 k-tiles
NC_ = 8              # cores

F32 = mybir.dt.float32
BF16 = mybir.dt.bfloat16
EXP = mybir.ActivationFunctionType.Exp
INV_SQRT_D = 1.0 / float(np.sqrt(D))

N_WARM = int(os.environ.get("K_WARM", "2"))
MM_BUFS = int(os.environ.get("K_MM_BUFS", "2"))
O_BUFS = int(os.environ.get("K_O_BUFS", "2"))


def _build_program():
    nc = bacc.Bacc(
        "TRN2",
        target_bir_lowering=False,
        debug=False,
        enable_asserts=False,
        num_devices=NC_,
    )
    qst = nc.dram_tensor("qst", (D, SQ), BF16, kind="ExternalInput").ap()
    kst = nc.dram_tensor("kst", (D, SK), BF16, kind="ExternalInput").ap()
    vsn = nc.dram_tensor("vsn", (SK, D), BF16, kind="ExternalInput").ap()
    wa = nc.dram_tensor("wa", (D, D), BF16, kind="ExternalInput").ap()
    wv = nc.dram_tensor("wv", (D, D), BF16, kind="ExternalInput").ap()
    out = nc.dram_tensor("out", (SQ, D), BF16, kind="ExternalOutput").ap()

    q_r = qst.rearrange("(it p) s -> p it s", p=P)
    k_r = kst.rearrange("(it p) s -> p it s", p=P)
    v_r = vsn.rearrange("(st p) d -> p st d", p=P)
    wa_r = wa.rearrange("(t p) n -> p t n", p=P)
    wv_r = wv.rearrange("(t p) n -> p t n", p=P)

    with tile.TileContext(nc) as tc, ExitStack() as ctx:
        sb = ctx.enter_context(tc.tile_pool(name="sb", bufs=1))
        ut_pool = ctx.enter_context(tc.tile_pool(name="ut", bufs=2))
        osb_pool = ctx.enter_context(tc.tile_pool(name="osb", bufs=4))
        mm_ps = ctx.enter_context(tc.tile_pool(name="mm", bufs=MM_BUFS, space="PSUM"))
        o_ps = ctx.enter_context(tc.tile_pool(name="o", bufs=2 * O_BUFS, space="PSUM"))
        rs_ps_pool = ctx.enter_context(tc.tile_pool(name="rs", bufs=1, space="PSUM"))
        warm_ps_pool = ctx.enter_context(tc.tile_pool(name="wm", bufs=1, space="PSUM"))

        qt_in = sb.tile([P, DT, SQ], BF16, tag="qt_in")
        wa_sb = sb.tile([P, DT, D], BF16, tag="wa")
        kt_sb = sb.tile([P, DT, SK], BF16, tag="kt")
        v_sb = sb.tile([P, KT, D], BF16, tag="v")
        wv_sb = sb.tile([P, DT, D], BF16, tag="wv")
        qpt_sb = sb.tile([P, DT, SQ], BF16, tag="qpt")
        pt_sb = sb.tile([P, KT, 512], BF16, tag="pt")
        ones_sb = sb.tile([P, 1], BF16, tag="ones")
        warm_sb = sb.tile([P, 512], BF16, tag="warm")
        rec_sb = sb.tile([P, 8], F32, tag="rec")

        rs_ps = rs_ps_pool.tile([P, 8], F32, tag="rs")
        warm_ps = warm_ps_pool.tile([P, 512], F32, tag="wm")

        # -- warm-up: pins the PE p-state ramp from ~1.2us (gpsimd memset
        # latency; the DVE queue's init drain is longer).
        nc.gpsimd.memset(warm_sb[:], 0.0)
        nc.gpsimd.memset(ones_sb[:], 1.0)
        nc.vector.memset(rs_ps[:], 0.0)
        for i in range(N_WARM):
            col = (i % 2) * 256
            nc.tensor.matmul(
                warm_ps[:, col : col + 256], warm_sb[:, 0:P],
                warm_sb[:, col : col + 256], start=True, stop=True,
            )

        # -- input DMA --
        # Transfers serialize on the DMA engines (~360 GB/s) and the SP
        # (HWDGE) queue is served predictably, so every input goes on SP in
        # exact first-consumption order: wa / qT interleaved to feed the Q'
        # sub-chunks as they start, then kT in S order, then v, then Wv.
        nc.sync.dma_start(wa_sb[:, 0:4, 0:256], wa_r[:, 0:4, 0:256])
        nc.sync.dma_start(qt_in[:, 0:4, 0:256], q_r[:, 0:4, 0:256])
        nc.sync.dma_start(wa_sb[:, 4:DT, 0:256], wa_r[:, 4:DT, 0:256])
        nc.sync.dma_start(qt_in[:, 4:DT, 0:256], q_r[:, 4:DT, 0:256])
        nc.sync.dma_start(wa_sb[:, :, 256:512], wa_r[:, :, 256:512])
        nc.sync.dma_start(wa_sb[:, :, 512:768], wa_r[:, :, 512:768])
        nc.sync.dma_start(wa_sb[:, :, 768:1024], wa_r[:, :, 768:1024])
        nc.sync.dma_start(qt_in[:, :, 256:512], q_r[:, :, 256:512])
        nc.sync.dma_start(qt_in[:, :, 512:1024], q_r[:, :, 512:1024])
        for g in range(SK // 512):
            nc.sync.dma_start(kt_sb[:, :, g * 512 : (g + 1) * 512],
                              k_r[:, :, g * 512 : (g + 1) * 512])
        nc.sync.dma_start(v_sb[:, 0:8, :], v_r[:, 0:8, :])
        nc.sync.dma_start(v_sb[:, 8:16, :], v_r[:, 8:16, :])
        nc.sync.dma_start(wv_sb[:], wv_r[:])

        # -- Q' phase: Q'^T[jt, cols] = sum_it A[it, jt-block]^T qT[it, cols]
        # in sub-chunks sized to the DMA arrival schedule --
        ncopy = 0
        for sub, (lo, hi) in enumerate(((0, 256), (256, 512), (512, 1024))):
            for jt in range(DT):
                qp = mm_ps.tile([P, 512], F32, tag="mm")
                for it in range(DT):
                    nc.tensor.matmul(
                        qp[:, 0 : hi - lo],
                        wa_sb[:, it, jt * P : (jt + 1) * P],
                        qt_in[:, it, lo:hi],
                        start=(it == 0),
                        stop=(it == DT - 1),
                    )
                dst = qpt_sb[:, jt, lo:hi]
                if ncopy % 2 == 0:
                    nc.scalar.copy(dst, qp[:, 0 : hi - lo])
                else:
                    nc.vector.tensor_copy(dst, qp[:, 0 : hi - lo])
                ncopy += 1

        def s_group(c, st):
            sps = mm_ps.tile([P, 512], F32, tag="mm")
            for it in range(DT):
                nc.tensor.matmul(
                    sps[:],
                    kt_sb[:, it, st * P : (st + 1) * P],
                    qpt_sb[:, it, c * 512 : (c + 1) * 512],
                    start=(it == 0),
                    stop=(it == DT - 1),
                )
            nc.scalar.activation(pt_sb[:, st, :], sps[:], EXP, scale=INV_SQRT_D)

        def rs_elems(c, st):
            # ap=1 accumulation chains: rs[:, c*4+qt] += PT[st][:, qtile]^T @ 1.
            # start=False always: matmul start_tensor_calc zeroes the whole
            # PSUM bank, which would erase the sibling chains sharing this
            # bank -- the bank is zeroed once by a memset instead.
            for qt_i in range(4):
                nc.tensor.matmul(
                    rs_ps[:, c * 4 + qt_i : c * 4 + qt_i + 1],
                    pt_sb[:, st, qt_i * P : (qt_i + 1) * P],
                    ones_sb[:],
                    start=False,
                    stop=(st == KT - 1),
                    skip_group_check=True,
                )

        def u_group(c, dt_i, ut):
            ups = mm_ps.tile([P, 512], F32, tag="mm")
            for st in range(KT):
                nc.tensor.matmul(
                    ups[:],
                    v_sb[:, st, dt_i * P : (dt_i + 1) * P],
                    pt_sb[:, st, :],
                    start=(st == 0),
                    stop=(st == KT - 1),
                )
            dst = ut[:, dt_i, :]
            if dt_i % 2 == 0:
                nc.scalar.copy(dst, ups[:])
            else:
                nc.vector.tensor_copy(dst, ups[:])

        def o_phase(c, ut):
            nc.vector.reciprocal(rec_sb[:, c * 4 : (c + 1) * 4],
                                 rs_ps[:, c * 4 : (c + 1) * 4])
            for qt_i in range(4):
                t = c * 4 + qt_i
                osb = osb_pool.tile([P, D], BF16, tag="osb")
                rec = rec_sb[:, t : t + 1]
                # each piece gets its own PSUM tile (finer release) and its
                # normalize+store fires as soon as its chain completes,
                # overlapping the next piece's matmuls. Stores alternate
                # queues so the final ones don't sit behind one queue's
                # per-DMA latency backlog. The very last tile tapers
                # (512/384/128) so the final mul+store chain is short.
                last = c == 1 and qt_i == 3
                pieces = ((0, 512), (512, 896), (896, 1024)) if last else (
                    (0, 512), (512, 1024))
                for h, (lo, hi) in enumerate(pieces):
                    ops = o_ps.tile([P, 512], F32, tag="o")
                    for i in range(DT):
                        nc.tensor.matmul(
                            ops[:, 0 : hi - lo],
                            ut[:, i, qt_i * P : (qt_i + 1) * P],
                            wv_sb[:, i, lo:hi],
                            start=(i == 0),
                            stop=(i == DT - 1),
                        )
                    dst = osb[:, lo:hi]
                    if (qt_i + h) % 2 == 0:
                        nc.scalar.mul(dst, ops[:, 0 : hi - lo], rec)
                        dma_eng = nc.sync
                    else:
                        nc.vector.tensor_scalar_mul(dst, ops[:, 0 : hi - lo], rec)
                        dma_eng = nc.scalar if c == 1 else nc.gpsimd
                    dma_eng.dma_start(out[t * P : (t + 1) * P, lo:hi], dst)

        # -- attention: S0 | U0+rs0 | S1 | O0 | U1+rs1 | O1 --
        ut0 = ut_pool.tile([P, DT, 512], BF16, tag="ut")
        for st in range(KT):
            s_group(0, st)
            if st > 0:
                rs_elems(0, st - 1)
        for dt_i in range(DT):
            u_group(0, dt_i, ut0)
            if dt_i == 0:
                rs_elems(0, KT - 1)

        ut1 = ut_pool.tile([P, DT, 512], BF16, tag="ut")
        for st in range(KT):
            s_group(1, st)
            if st > 0:
                rs_elems(1, st - 1)
        o_phase(0, ut0)
        for dt_i in range(DT):
            u_group(1, dt_i, ut1)
            if dt_i == 0:
                rs_elems(1, KT - 1)
        o_phase(1, ut1)

    nc.compile()
    return nc


_NC_CACHE = {}


def _get_nc():
    if "nc" not in _NC_CACHE:
        _NC_CACHE["nc"] = _build_program()
    return _NC_CACHE["nc"]


def _numpy_fallback(q, k, v, Wq, bq, Wk, bk, Wv, bv):
    out = np.empty((B, S, D), np.float32)
    for b in range(B):
        qp = q[b] @ Wq + bq
        kp = k[b] @ Wk + bk
        vpv = v[b] @ Wv + bv
        s = (qp @ kp.T) * INV_SQRT_D
        s -= s.max(axis=-1, keepdims=True)
        p = np.exp(s)
        p /= p.sum(axis=-1, keepdims=True)
        out[b] = p @ vpv
    return out


def kernel(q, k, v, Wq, bq, Wk, bk, Wv, bv):
    from ml_dtypes import bfloat16

    q = np.asarray(q, np.float32)
    k = np.asarray(k, np.float32)
    v = np.asarray(v, np.float32)
    Wq = np.ascontiguousarray(np.asarray(Wq, np.float32))
    Wk = np.ascontiguousarray(np.asarray(Wk, np.float32))
    Wv = np.ascontiguousarray(np.asarray(Wv, np.float32))
    bq = np.asarray(bq, np.float32)
    bk = np.asarray(bk, np.float32)
    bv = np.asarray(bv, np.float32)

    if np.any(bq) or np.any(bk) or np.any(bv):
        # Never hit for this problem (biases are structurally zero), kept for
        # exactness of the kernel contract.
        return _numpy_fallback(q, k, v, Wq, bq, Wk, bk, Wv, bv)

    nc = _get_nc()
    A = (Wq @ Wk.T).astype(bfloat16)         # scores = q A k^T
    wv_b = Wv.astype(bfloat16)
    kt_full = [np.ascontiguousarray(k[b].T.astype(bfloat16)) for b in range(B)]
    v_full = [np.ascontiguousarray(v[b].astype(bfloat16)) for b in range(B)]
    in_maps = []
    for b in range(B):
        for h in range(2):
            in_maps.append(
                {
                    "qst": np.ascontiguousarray(
                        q[b, h * SQ : (h + 1) * SQ, :].T.astype(bfloat16)
                    ),
                    "kst": kt_full[b],
                    "vsn": v_full[b],
                    "wa": A,
                    "wv": wv_b,
                }
            )

    res = bass_utils.run_bass_kernel_spmd(nc, in_maps, core_ids=list(range(NC_)))

    out = np.empty((B, S, D), np.float32)
    for c, r in enumerate(res.results):
        b, h = divmod(c, 2)
        out[b, h * SQ : (h + 1) * SQ, :] = np.asarray(r["out"], np.float32)
    return out


if __name__ == "__main__":
    rng = np.random.default_rng(0)
    scale = 1.0 / np.sqrt(D)
    inputs = {
        "q": rng.standard_normal((B, S, D)).astype(np.float32),
        "k": rng.standard_normal((B, S, D)).astype(np.float32),
        "v": rng.standard_normal((B, S, D)).astype(np.float32),
        "Wq": (rng.standard_normal((D, D)) * scale).astype(np.float32),
        "bq": np.zeros(D, np.float32),
        "Wk": (rng.standard_normal((D, D)) * scale).astype(np.float32),
        "bk": np.zeros(D, np.float32),
        "Wv": (rng.standard_normal((D, D)) * scale).astype(np.float32),
        "bv": np.zeros(D, np.float32),
    }
    actual = kernel(**inputs)
    expected = _numpy_fallback(**inputs)
    err = np.linalg.norm(actual - expected) / np.linalg.norm(expected)
    print("rel err:", err)


# revision 42
# speedup vs baseline: 1.0065x; 1.0015x over previous
"""Trainium2 Bass kernel for single-headed attention.

Problem: nn_Attention_17471926960981
  q,k,v: [4, 2048, 1024] f32; Wq/Wk/Wv: [1024,1024]; bq/bk/bv: [1024] (zeros)
  out = softmax((q@Wq)(k@Wk)^T / sqrt(1024)) @ (v@Wv)   per batch item

Sharding: 8 cores = (batch b in 0..3, seq-half h in 0..1). Each core gets
1024 rows of q for its batch item plus the full k/v of that item and
computes its 1024 output rows independently.

Algebraic restructure (associativity; host does the cheap 1024^3 prep):
  scores = (q Wq)(k Wk)^T = q A k^T          with A = Wq Wk^T (host sgemm)
  out    = P (v Wv)       = (P v) Wv
so the device never projects k or v.

v2 layout (all matmul operands bf16, ~0.5% rel err vs the 2e-2 gate;
PSUM accumulation f32; measured 173.5us vs the 225.6us fp32r baseline):
  1. Q'^T [d, q] = A^T q^T, resident in SBUF (no DRAM spill), computed in
     column sub-chunks sized to the DMA arrival schedule.
  2. S^T computed directly in [k-part, q-free] orientation per 512-wide q
     chunk: stationary = kT d-block, moving = Q'^T. exp() on ACT writes
     P^T straight into the layout the U matmul needs -- no PE transposes,
     no transpose copies. Softmax is shift-invariant and scaled scores are
     O(1): no row-max pass.
  3. Row-sums of P via ap=1 accumulation chains: stationary = P^T block,
     moving = a ones column (memset, no DMA). Lands rs as [q-part, 1] with
     near-zero PE engine time. The four chains share one PSUM bank, and
     matmul start_tensor_calc zeroes the WHOLE bank on real hardware, so
     the bank is zeroed once by a DVE memset and every chain element runs
     start=False (pure accumulate).
  4. U^T [d, q] = v^T P^T; O = U^T.T Wv, normalized by 1/rowsum on the
     way out (ACT/DVE alternating), stored bf16 per 512-half as soon as
     each half's PSUM chain completes (host upcasts to f32).
  5. A 2-matmul PE warm-up on a memset tile pins the tensor-engine
     p-state ramp (an idle PE engine resets it, doubling the cost of the
     next ~3us of dispatches), all input DMA goes on the SP queue in
     exact first-consumption order, and phases are ordered (Q' c0,c1 |
     S0 | U0+rs0 | S1 | O0 | U1+rs1 | O1) so every consumer's inputs are
     produced a full phase ahead -- the PE engine runs gap-free at full
     rate mid-program (busy ~164.3us of a 163.8us matmul floor).

Biases are structurally zero in this problem; kernel() falls back to an
exact numpy path in the (never exercised) case they are nonzero.
"""

import os
import sys

import numpy as np

try:
    import concourse.bass as bass  # noqa: F401
except ImportError:  # pragma: no cover
    sys.path.insert(0, "/opt/trn_rl_repo")

from contextlib import ExitStack

import concourse.bass as bass  # noqa: F401
import concourse.bass_utils as bass_utils
import concourse.mybir as mybir
import concourse.tile as tile
from concourse import bacc

B, S, D = 4, 2048, 1024
P = 128
SQ = S // 2          # q rows per core
SK = S               # kv rows per core
DT = D // P          # 8 d-tiles
KT = SK // P         # 16 k-tiles
NC_ = 8              # cores

F32 = mybir.dt.float32
BF16 = mybir.dt.bfloat16
EXP = mybir.ActivationFunctionType.Exp
INV_SQRT_D = 1.0 / float(np.sqrt(D))

N_WARM = int(os.environ.get("K_WARM", "2"))
MM_BUFS = int(os.environ.get("K_MM_BUFS", "2"))
O_BUFS = int(os.environ.get("K_O_BUFS", "2"))


def _build_program():
    nc = bacc.Bacc(
        "TRN2",
        target_bir_lowering=False,
        debug=False,
        enable_asserts=False,
        num_devices=NC_,
    )
    qst = nc.dram_tensor("qst", (D, SQ), BF16, kind="ExternalInput").ap()
    kst = nc.dram_tensor("kst", (D, SK), BF16, kind="ExternalInput").ap()
    vsn = nc.dram_tensor("vsn", (SK, D), BF16, kind="ExternalInput").ap()
    wa = nc.dram_tensor("wa", (D, D), BF16, kind="ExternalInput").ap()
    wv = nc.dram_tensor("wv", (D, D), BF16, kind="ExternalInput").ap()
    out = nc.dram_tensor("out", (SQ, D), BF16, kind="ExternalOutput").ap()

    q_r = qst.rearrange("(it p) s -> p it s", p=P)
    k_r = kst.rearrange("(it p) s -> p it s", p=P)
    v_r = vsn.rearrange("(st p) d -> p st d", p=P)
    wa_r = wa.rearrange("(t p) n -> p t n", p=P)
    wv_r = wv.rearrange("(t p) n -> p t n", p=P)

    with tile.TileContext(nc) as tc, ExitStack() as ctx:
        sb = ctx.enter_context(tc.tile_pool(name="sb", bufs=1))
        ut_pool = ctx.enter_context(tc.tile_pool(name="ut", bufs=2))
        osb_pool = ctx.enter_context(tc.tile_pool(name="osb", bufs=4))
        mm_ps = ctx.enter_context(tc.tile_pool(name="mm", bufs=MM_BUFS, space="PSUM"))
        o_ps = ctx.enter_context(tc.tile_pool(name="o", bufs=2 * O_BUFS, space="PSUM"))
        rs_ps_pool = ctx.enter_context(tc.tile_pool(name="rs", bufs=1, space="PSUM"))
        warm_ps_pool = ctx.enter_context(tc.tile_pool(name="wm", bufs=1, space="PSUM"))

        qt_in = sb.tile([P, DT, SQ], BF16, tag="qt_in")
        wa_sb = sb.tile([P, DT, D], BF16, tag="wa")
        kt_sb = sb.tile([P, DT, SK], BF16, tag="kt")
        v_sb = sb.tile([P, KT, D], BF16, tag="v")
        wv_sb = sb.tile([P, DT, D], BF16, tag="wv")
        qpt_sb = sb.tile([P, DT, SQ], BF16, tag="qpt")
        pt_sb = sb.tile([P, KT, 512], BF16, tag="pt")
        ones_sb = sb.tile([P, 1], BF16, tag="ones")
        warm_sb = sb.tile([P, 512], BF16, tag="warm")
        rec_sb = sb.tile([P, 8], F32, tag="rec")

        rs_ps = rs_ps_pool.tile([P, 8], F32, tag="rs")
        warm_ps = warm_ps_pool.tile([P, 512], F32, tag="wm")

        # -- warm-up: pins the PE p-state ramp from ~1.2us (gpsimd memset
        # latency; the DVE queue's init drain is longer).
        nc.gpsimd.memset(warm_sb[:], 0.0)
        nc.gpsimd.memset(ones_sb[:], 1.0)
        nc.vector.memset(rs_ps[:], 0.0)
        for i in range(N_WARM):
            col = (i % 2) * 256
            nc.tensor.matmul(
                warm_ps[:, col : col + 256], warm_sb[:, 0:P],
                warm_sb[:, col : col + 256], start=True, stop=True,
            )

        # -- input DMA --
        # Transfers serialize on the DMA engines (~360 GB/s) and the SP
        # (HWDGE) queue is served predictably, so every input goes on SP in
        # exact first-consumption order: wa / qT interleaved to feed the Q'
        # sub-chunks as they start, then kT in S order, then v, then Wv.
        nc.sync.dma_start(wa_sb[:, 0:4, 0:256], wa_r[:, 0:4, 0:256])
        nc.sync.dma_start(qt_in[:, 0:4, 0:256], q_r[:, 0:4, 0:256])
        nc.sync.dma_start(wa_sb[:, 4:DT, 0:256], wa_r[:, 4:DT, 0:256])
        nc.sync.dma_start(qt_in[:, 4:DT, 0:256], q_r[:, 4:DT, 0:256])
        nc.sync.dma_start(wa_sb[:, :, 256:512], wa_r[:, :, 256:512])
        nc.sync.dma_start(wa_sb[:, :, 512:768], wa_r[:, :, 512:768])
        nc.sync.dma_start(wa_sb[:, :, 768:1024], wa_r[:, :, 768:1024])
        nc.sync.dma_start(qt_in[:, :, 256:512], q_r[:, :, 256:512])
        nc.sync.dma_start(qt_in[:, :, 512:1024], q_r[:, :, 512:1024])
        for g in range(SK // 512):
            nc.sync.dma_start(kt_sb[:, :, g * 512 : (g + 1) * 512],
                              k_r[:, :, g * 512 : (g + 1) * 512])
        nc.sync.dma_start(v_sb[:, 0:8, :], v_r[:, 0:8, :])
        nc.sync.dma_start(v_sb[:, 8:16, :], v_r[:, 8:16, :])
        nc.sync.dma_start(wv_sb[:], wv_r[:])

        # -- Q' phase: Q'^T[jt, cols] = sum_it A[it, jt-block]^T qT[it, cols]
        # in sub-chunks sized to the DMA arrival schedule --
        ncopy = 0
        for sub, (lo, hi) in enumerate(((0, 256), (256, 512), (512, 1024))):
            for jt in range(DT):
                qp = mm_ps.tile([P, 512], F32, tag="mm")
                for it in range(DT):
                    nc.tensor.matmul(
                        qp[:, 0 : hi - lo],
                        wa_sb[:, it, jt * P : (jt + 1) * P],
                        qt_in[:, it, lo:hi],
                        start=(it == 0),
                        stop=(it == DT - 1),
                    )
                dst = qpt_sb[:, jt, lo:hi]
                if ncopy % 2 == 0:
                    nc.scalar.copy(dst, qp[:, 0 : hi - lo])
                else:
                    nc.vector.tensor_copy(dst, qp[:, 0 : hi - lo])
                ncopy += 1

        def s_group(c, st):
            sps = mm_ps.tile([P, 512], F32, tag="mm")
            for it in range(DT):
                nc.tensor.matmul(
                    sps[:],
                    kt_sb[:, it, st * P : (st + 1) * P],
                    qpt_sb[:, it, c * 512 : (c + 1) * 512],
                    start=(it == 0),
                    stop=(it == DT - 1),
                )
            nc.scalar.activation(pt_sb[:, st, :], sps[:], EXP, scale=INV_SQRT_D)

        def rs_elems(c, st):
            # ap=1 accumulation chains: rs[:, c*4+qt] += PT[st][:, qtile]^T @ 1.
            # start=False always: matmul start_tensor_calc zeroes the whole
            # PSUM bank, which would erase the sibling chains sharing this
            # bank -- the bank is zeroed once by a memset instead.
            for qt_i in range(4):
                nc.tensor.matmul(
                    rs_ps[:, c * 4 + qt_i : c * 4 + qt_i + 1],
                    pt_sb[:, st, qt_i * P : (qt_i + 1) * P],
                    ones_sb[:],
                    start=False,
                    stop=(st == KT - 1),
                    skip_group_check=True,
                )

        def u_group(c, dt_i, ut):
            ups = mm_ps.tile([P, 512], F32, tag="mm")
            for st in range(KT):
                nc.tensor.matmul(
                    ups[:],
                    v_sb[:, st, dt_i * P : (dt_i + 1) * P],
                    pt_sb[:, st, :],
                    start=(st == 0),
                    stop=(st == KT - 1),
                )
            dst = ut[:, dt_i, :]
            if dt_i % 2 == 0:
                nc.scalar.copy(dst, ups[:])
            else:
                nc.vector.tensor_copy(dst, ups[:])

        def o_phase(c, ut):
            nc.vector.reciprocal(rec_sb[:, c * 4 : (c + 1) * 4],
                                 rs_ps[:, c * 4 : (c + 1) * 4])
            for qt_i in range(4):
                t = c * 4 + qt_i
                osb = osb_pool.tile([P, D], BF16, tag="osb")
                rec = rec_sb[:, t : t + 1]
                # each piece gets its own PSUM tile (finer release) and its
                # normalize+store fires as soon as its chain completes,
                # overlapping the next piece's matmuls. Stores alternate
                # queues so the final ones don't sit behind one queue's
                # per-DMA latency backlog. The very last tile tapers
                # (512/384/128) so the final mul+store chain is short.
                pieces = ((0, 512), (512, 1024))
                for h, (lo, hi) in enumerate(pieces):
                    ops = o_ps.tile([P, 512], F32, tag="o")
                    for i in range(DT):
                        nc.tensor.matmul(
                            ops[:, 0 : hi - lo],
                            ut[:, i, qt_i * P : (qt_i + 1) * P],
                            wv_sb[:, i, lo:hi],
                            start=(i == 0),
                            stop=(i == DT - 1),
                        )
                    dst = osb[:, lo:hi]
                    if (qt_i + h) % 2 == 0:
                        nc.scalar.mul(dst, ops[:, 0 : hi - lo], rec)
                        dma_eng = nc.sync
                    else:
                        nc.vector.tensor_scalar_mul(dst, ops[:, 0 : hi - lo], rec)
                        dma_eng = nc.scalar if c == 1 else nc.gpsimd
                    dma_eng.dma_start(out[t * P : (t + 1) * P, lo:hi], dst)

        # -- attention: S0 | U0+rs0 | S1 | O0 | U1+rs1 | O1 --
        ut0 = ut_pool.tile([P, DT, 512], BF16, tag="ut")
        for st in range(KT):
            s_group(0, st)
            if st > 0:
                rs_elems(0, st - 1)
        for dt_i in range(DT):
            u_group(0, dt_i, ut0)
            if dt_i == 0:
                rs_elems(0, KT - 1)

        ut1 = ut_pool.tile([P, DT, 512], BF16, tag="ut")
        for st in range(KT):
            s_group(1, st)
            if st > 0:
                rs_elems(1, st - 1)
        o_phase(0, ut0)
        for dt_i in range(DT):
            u_group(1, dt_i, ut1)
            if dt_i == 0:
                rs_elems(1, KT - 1)
        o_phase(1, ut1)

    nc.compile()
    return nc


_NC_CACHE = {}


def _get_nc():
    if "nc" not in _NC_CACHE:
        _NC_CACHE["nc"] = _build_program()
    return _NC_CACHE["nc"]


def _numpy_fallback(q, k, v, Wq, bq, Wk, bk, Wv, bv):
    out = np.empty((B, S, D), np.float32)
    for b in range(B):
        qp = q[b] @ Wq + bq
        kp = k[b] @ Wk + bk
        vpv = v[b] @ Wv + bv
        s = (qp @ kp.T) * INV_SQRT_D
        s -= s.max(axis=-1, keepdims=True)
        p = np.exp(s)
        p /= p.sum(axis=-1, keepdims=True)
        out[b] = p @ vpv
    return out


def kernel(q, k, v, Wq, bq, Wk, bk, Wv, bv):
    from ml_dtypes import bfloat16

    q = np.asarray(q, np.float32)
    k = np.asarray(k, np.float32)
    v = np.asarray(v, np.float32)
    Wq = np.ascontiguousarray(np.asarray(Wq, np.float32))
    Wk = np.ascontiguousarray(np.asarray(Wk, np.float32))
    Wv = np.ascontiguousarray(np.asarray(Wv, np.float32))
    bq = np.asarray(bq, np.float32)
    bk = np.asarray(bk, np.float32)
    bv = np.asarray(bv, np.float32)

    if np.any(bq) or np.any(bk) or np.any(bv):
        # Never hit for this problem (biases are structurally zero), kept for
        # exactness of the kernel contract.
        return _numpy_fallback(q, k, v, Wq, bq, Wk, bk, Wv, bv)

    nc = _get_nc()
    A = (Wq @ Wk.T).astype(bfloat16)         # scores = q A k^T
    wv_b = Wv.astype(bfloat16)
    kt_full = [np.ascontiguousarray(k[b].T.astype(bfloat16)) for b in range(B)]
    v_full = [np.ascontiguousarray(v[b].astype(bfloat16)) for b in range(B)]
    in_maps = []
    for b in range(B):
        for h in range(2):
            in_maps.append(
                {
                    "qst": np.ascontiguousarray(
                        q[b, h * SQ : (h + 1) * SQ, :].T.astype(bfloat16)
                    ),
                    "kst": kt_full[b],
                    "vsn": v_full[b],
                    "wa": A,
                    "wv": wv_b,
                }
            )

    res = bass_utils.run_bass_kernel_spmd(nc, in_maps, core_ids=list(range(NC_)))

    out = np.empty((B, S, D), np.float32)
    for c, r in enumerate(res.results):
        b, h = divmod(c, 2)
        out[b, h * SQ : (h + 1) * SQ, :] = np.asarray(r["out"], np.float32)
    return out


if __name__ == "__main__":
    rng = np.random.default_rng(0)
    scale = 1.0 / np.sqrt(D)
    inputs = {
        "q": rng.standard_normal((B, S, D)).astype(np.float32),
        "k": rng.standard_normal((B, S, D)).astype(np.float32),
        "v": rng.standard_normal((B, S, D)).astype(np.float32),
        "Wq": (rng.standard_normal((D, D)) * scale).astype(np.float32),
        "bq": np.zeros(D, np.float32),
        "Wk": (rng.standard_normal((D, D)) * scale).astype(np.float32),
        "bk": np.zeros(D, np.float32),
        "Wv": (rng.standard_normal((D, D)) * scale).astype(np.float32),
        "bv": np.zeros(D, np.float32),
    }
    actual = kernel(**inputs)
    expected = _numpy_fallback(**inputs)
    err = np.linalg.norm(actual - expected) / np.linalg.norm(expected)
    print("rel err:", err)
